# revision 18
# baseline (speedup 1.0000x reference)
"""Causal self-attention (B=2, T=2048, C=1024, H=16) on 8 Trainium2 cores.

Sharding: data-parallel over batch (2) x tensor-parallel over heads (4 groups
of 4 heads). Core c handles batch b = c//4, head group g = c%4 (heads 4g..4g+3).
Each core computes its qkv column slice, full causal TxT attention for its 4
heads, and a partial row-parallel projection. Host sums the 4 partial proj
outputs per batch and adds b_proj.

v4 layout notes (on top of the v3 design):
- PSUM partitioned so the score double-buffer is never polluted:
  "s" scores [128,2,512] f32 x2 bufs (4 banks), av0/av1 accumulators x1 buf
  (2 banks), and a shared transient pool "p" [128,512] f32 x2 bufs (2 banks)
  that qkv/v/proj/bcast matmuls rotate through. In v3 proj tiles rotated
  through the score tag, which made every score matmul wait on the previous
  chunk's exp drain (measured 547ns/chunk of ACT idle).
- av bufs=1: at block end av0/av1 are drained to SBUF (yU tiles, f16,
  unnormalized + embedded den rows) split across DVE and GpSimd so the next
  block's first AV accumulate (start=True) finds the banks free.
- normalization (lagged one block): den rows -> dsb, sel-matrix broadcast
  matmul (into "p"), reciprocal_approx_fast, two [64,512] multiplies reading
  the SBUF yU tiles into yT.
- chunks processed in PAIRS: score(jc), score(jc+1) back-to-back, then both
  AV quads chained. Halves the PE 64<->128-row mode switches (~107ns each)
  and keeps ACT exps back-to-back.
- proj psum->sbuf copies ride GpSimd; DVE keeps bias adds, tri-mask, drain
  copies, reciprocal, norm multiplies.
- DMA: cc-granular first transfers so the first qkv chain starts ~4us in.
"""

import os
import sys

sys.path.insert(0, "/opt/trn_rl_repo")

import numpy as np

P = 128
T = 2048
C = 1024
D = 64
HPC = 4          # heads per core
HD = HPC * D     # 256 qkv columns per core
CC = C // P      # 8 contraction chunks
TC = T // P      # 16 t-chunks of 128
IC = T // 512    # 4 i-chunks of 512

# const blob column offsets
OFF_TRI = 0
OFF_SEL = 128
OFF_BQ = 256
OFF_BK = 258
OFF_BV = 260
CSTW = 516

_NC = None
LAST_RESULTS = None


def _build_nc():
    import concourse.mybir as mybir
    import concourse.tile as tile
    from concourse import bacc
    from contextlib import ExitStack

    dt = mybir.dt
    f32 = dt.float32
    f16 = dt.float16
    ALU = mybir.AluOpType
    ACTF = mybir.ActivationFunctionType

    nc = bacc.Bacc(
        "TRN2",
        target_bir_lowering=False,
        debug=False,
        enable_asserts=False,
        num_devices=8,
    )

    # host-packed layouts: contiguous per-partition lines per transfer
    xq = nc.dram_tensor("xq", [P, 4, CC, 512], f16, kind="ExternalInput").ap()
    wq2 = nc.dram_tensor("wq2", [P, 2, CC, P], f16, kind="ExternalInput").ap()
    wk2 = nc.dram_tensor("wk2", [P, 2, CC, P], f16, kind="ExternalInput").ap()
    wv2 = nc.dram_tensor("wv2", [P, CC, HD], f16, kind="ExternalInput").ap()
    wp2 = nc.dram_tensor("wp2", [P, 2, C], f16, kind="ExternalInput").ap()
    cst = nc.dram_tensor("cst", [P, CSTW], f16, kind="ExternalInput").ap()
    out = nc.dram_tensor("out", [T, C], f16, kind="ExternalOutput").ap()

    with tile.TileContext(nc) as tc, ExitStack() as ctx:
        persist = ctx.enter_context(tc.tile_pool(name="persist", bufs=1))
        qT_sb = persist.tile([P, 2, T], f16, name="qT")    # [d%128, dchunk, t]
        kT_sb = persist.tile([P, 2, T], f16, name="kT")
        v_sb = persist.tile([P, TC, 2, 2, P], f16, name="v")  # [t%128, tchunk, hpair, hi, 128-padded d]
        yT_sb = persist.tile([P, 2, T], f16, name="yT")
        wp_sb = persist.tile([P, 2, C], f16, name="wps")
        cst_sb = persist.tile([P, CSTW], f16, name="csts")
        dsb = persist.tile([P, 512], f16, name="dsb")
        xs_sb = persist.tile([P, 4, CC, 512], f16, name="xss")
        wq_sb = persist.tile([P, 2, CC, P], f16, name="wqs")
        wk_sb = persist.tile([P, 2, CC, P], f16, name="wks")
        wv_sb = persist.tile([P, CC, HD], f16, name="wvs")

        tri_v = cst_sb[:, OFF_TRI:OFF_TRI + P]
        sel_v = cst_sb[:, OFF_SEL:OFF_SEL + P]
        bq_v = cst_sb[:, OFF_BQ:OFF_BQ + 2]
        bk_v = cst_sb[:, OFF_BK:OFF_BK + 2]
        bv_v = cst_sb[:, OFF_BV:OFF_BV + HD].rearrange(
            "p (hp hi d) -> p hp hi d", hi=2, d=D
        )

        ph2 = ctx.enter_context(tc.tile_pool(name="ph2", bufs=5))
        ph3 = ctx.enter_context(tc.tile_pool(name="ph3", bufs=3))
        pu = ctx.enter_context(tc.tile_pool(name="pu", bufs=2))
        ps_s = ctx.enter_context(tc.tile_pool(name="ps_s", bufs=2, space="PSUM"))
        ps_av = ctx.enter_context(tc.tile_pool(name="ps_av", bufs=1, space="PSUM"))
        ps_p = ctx.enter_context(tc.tile_pool(name="ps_p", bufs=2, space="PSUM"))

        # one-time inits (no inputs needed)
        nc.vector.memset(dsb[:, :], 0.0)
        # constant regions of the padded v operand: zeros + the ones column
        # that makes each AV matmul emit the softmax denominator row
        nc.vector.memset(v_sb[:, :, :, 0, D + 1:P], 0.0)
        nc.vector.memset(v_sb[:, :, :, 1, 1:D], 0.0)
        nc.vector.memset(v_sb[:, :, :, 0, D:D + 1], 1.0)
        nc.vector.memset(v_sb[:, :, :, 1, 0:1], 1.0)

        # ---------------- DMA, first-needed-first ----------------
        # each dma_start costs ~0.6-0.8us of sequencer descriptor-generation
        # (DIRECT2D), so keep descriptors COARSE and spread across the three
        # trigger queues; one transfer fans out over all 16 DMA engines
        nc.sync.dma_start(xs_sb[:, 0, 0:1, :], xq[:, 0, 0:1, :])
        nc.scalar.dma_start(wq_sb[:, 0, :, :], wq2[:, 0, :, :])
        nc.gpsimd.dma_start(wk_sb[:, 0, :, :], wk2[:, 0, :, :])
        nc.sync.dma_start(xs_sb[:, 0, 1:4, :], xq[:, 0, 1:4, :])
        nc.sync.dma_start(xs_sb[:, 0, 4:8, :], xq[:, 0, 4:8, :])
        nc.scalar.dma_start(wq_sb[:, 1, :, :], wq2[:, 1, :, :])
        nc.gpsimd.dma_start(wv_sb[:], wv2)
        nc.sync.dma_start(cst_sb[:], cst)
        nc.gpsimd.dma_start(wk_sb[:, 1, :, :], wk2[:, 1, :, :])
        for tq in range(1, 4):
            nc.sync.dma_start(xs_sb[:, tq, 0:4, :], xq[:, tq, 0:4, :])
            nc.gpsimd.dma_start(xs_sb[:, tq, 4:8, :], xq[:, tq, 4:8, :])
        nc.gpsimd.dma_start(wp_sb[:], wp2)

        # ---------------- phase 1 units ----------------
        def emit_qk(W_s, bco, dest, co, tsl):
            ps = ps_p.tile([P, 512], f32, tag="p", name=f"qkp{co}_{tsl}")
            for cc in range(CC):
                nc.tensor.matmul(
                    ps[:, :],
                    W_s[:, co, cc, :],
                    xs_sb[:, tsl, cc, :],
                    start=(cc == 0),
                    stop=(cc == CC - 1),
                )
            nc.vector.tensor_tensor(
                dest[:, co, tsl * 512:(tsl + 1) * 512],
                ps[:, :],
                bco.to_broadcast([P, 512]),
                ALU.add,
            )

        def emit_v(tj):
            ps = ps_p.tile([P, 512], f32, tag="p", name=f"vp{tj}")
            for cc in range(CC):
                nc.tensor.matmul(
                    ps[:, 0:HD],
                    xs_sb[:, tj // 4, cc, (tj % 4) * P:(tj % 4 + 1) * P],
                    wv_sb[:, cc, :],
                    start=(cc == 0),
                    stop=(cc == CC - 1),
                )
            psv = ps[:, 0:HD].rearrange("p (hp hi d) -> p hp hi d", hi=2, d=D)
            nc.vector.tensor_tensor(
                v_sb[:, tj, :, 0, 0:D], psv[:, :, 0, :], bv_v[:, :, 0, :], ALU.add
            )
            nc.vector.tensor_tensor(
                v_sb[:, tj, :, 1, D:P], psv[:, :, 1, :], bv_v[:, :, 1, :], ALU.add
            )

        # prologue: quarter 0 of everything (needed before block ci=0)
        for co in range(2):
            emit_qk(wq_sb, cst_sb[:, OFF_BQ + co:OFF_BQ + co + 1], qT_sb, co, 0)
            emit_qk(wk_sb, cst_sb[:, OFF_BK + co:OFF_BK + co + 1], kT_sb, co, 0)
        for tj in range(4):
            emit_v(tj)

        # quarters 1-3 become attention-phase PE filler, tagged with the
        # t-quarter so blocks that need them can force-drain first
        ph1_fill = []
        for tsl in (1, 2, 3):
            for W_s, boff, dest, kind in (
                (wq_sb, OFF_BQ, qT_sb, "q"),
                (wk_sb, OFF_BK, kT_sb, "k"),
            ):
                for co in range(2):
                    ph1_fill.append((tsl, kind, 1950.0, lambda
                        W_s=W_s, boff=boff, dest=dest, co=co, tsl=tsl:
                        emit_qk(
                            W_s, cst_sb[:, boff + co:boff + co + 1],
                            dest, co, tsl,
                        )))
            for tj in range(4 * tsl, 4 * tsl + 4):
                ph1_fill.append((tsl, "v", 1300.0, lambda tj=tj: emit_v(tj)))
        proj_fill = []

        # ---------------- phase 2: attention + woven proj ----------------
        def emit_proj_tj(tj, tail=False):
            ot = ph3.tile([P, C], f16, tag="ot", name=f"ot{tj}")
            for co in range(2):
                # in the tail the score banks are free: a 4-slot psum
                # rotation keeps the proj matmuls ahead of the copies
                if tail and co == 0:
                    pps = ps_s.tile([P, 512], f32, tag="s", name=f"pp{tj}_{co}")
                else:
                    pps = ps_p.tile([P, 512], f32, tag="p", name=f"pp{tj}_{co}")
                for dc in range(2):
                    nc.tensor.matmul(
                        pps[:, :],
                        yT_sb[:, dc, tj * P:(tj + 1) * P],
                        wp_sb[:, dc, co * 512:(co + 1) * 512],
                        start=(dc == 0),
                        stop=(dc == 1),
                    )
                # in the tail ACT has no exps left: split the copies so the
                # last tiles drain twice as fast, and DMA each half out as
                # soon as its copy lands
                if tail and co == 1:
                    nc.scalar.copy(ot[:, co * 512:(co + 1) * 512], pps[:, :])
                else:
                    nc.vector.tensor_copy(
                        ot[:, co * 512:(co + 1) * 512], pps[:, :]
                    )
                if tail:
                    nc.sync.dma_start(
                        out[tj * P:(tj + 1) * P, co * 512:(co + 1) * 512],
                        ot[:, co * 512:(co + 1) * 512],
                    )
            if not tail:
                nc.sync.dma_start(out[tj * P:(tj + 1) * P, :], ot[:])

        # emitted-work accounting (ns) to pace fillers: the PE stream should
        # stay at least as long as the ACT (exp) stream it depends on
        clk = {"pe": 0.0, "act": 0.0}

        def fill(n, max_q=4):
            # max_q: don't pull qkv chains whose x-quarter DMA is still in
            # flight — a DMA-blocked filler matmul stalls the in-order PE
            # queue ahead of the next score matmuls
            for _ in range(n):
                if ph1_fill and ph1_fill[0][0] <= max_q:
                    q, kind, est, fn = ph1_fill.pop(0)
                    fn()
                    clk["pe"] += est
                elif proj_fill:
                    proj_fill.pop(0)()
                    clk["pe"] += 1250.0
                else:
                    return

        def fill_to_rate(max_q=4):
            while (ph1_fill or proj_fill) and clk["act"] > clk["pe"]:
                n0 = len(ph1_fill) + len(proj_fill)
                fill(1, max_q=max_q)
                if len(ph1_fill) + len(proj_fill) == n0:
                    return

        def drain_ph1(upto_quarter, kinds=("q", "k", "v")):
            i = 0
            while i < len(ph1_fill):
                q, kind, est, fn = ph1_fill[i]
                if q <= upto_quarter and kind in kinds:
                    ph1_fill.pop(i)
                    fn()
                    clk["pe"] += est
                else:
                    i += 1

        def emit_norm(hp, i0, yU0, yU1):
            # den rows were copied into dsb at the block's end; broadcast +
            # reciprocal + scale the SBUF yU tiles into yT
            bps = ps_p.tile([P, 512], f32, tag="p", name="bps")
            nc.tensor.matmul(
                bps[:, :], sel_v, dsb[:],
                start=True, stop=True, skip_group_check=True,
            )
            rec = ph2.tile([P, 512], f32, tag="rec", bufs=2, name="rec")
            nc.vector.reciprocal_approx_fast(rec[:, :], bps[:, :])
            nc.gpsimd.tensor_tensor(
                yT_sb[0:D, hp, i0:i0 + 512], yU0[0:D, :], rec[0:D, :], ALU.mult
            )
            nc.gpsimd.tensor_tensor(
                yT_sb[D:P, hp, i0:i0 + 512], yU1[D:P, :], rec[D:P, :], ALU.mult
            )
            if hp == 1 and i0 < 1536:
                ci = i0 // 512
                for tj in range(4 * ci, 4 * ci + 4):
                    proj_fill.append(lambda tj=tj: emit_proj_tj(tj))

        pending = None
        for ci in range(IC):
            i0 = ci * 512
            njc = 4 * (ci + 1)
            if ci >= 1:
                # only this i-chunk's qT is needed before the block starts;
                # its kT/v stragglers can drain any time before the diagonal
                drain_ph1(ci, kinds=("q",))
            for hp in range(2):
                av0 = ps_av.tile([P, 512], f32, tag="av0", name="av0")
                av1 = ps_av.tile([P, 512], f32, tag="av1", name="av1")

                def emit_s(jc):
                    diag = jc >= 4 * ci
                    o = (jc - 4 * ci) if diag else 0
                    c0 = o * P
                    sps = ps_s.tile([P, 2, 512], f32, tag="s", name="sps")
                    for hi in range(2):
                        bp = D * hi
                        nc.tensor.matmul(
                            sps[:, hi, c0:512],
                            kT_sb[bp:bp + D, hp, jc * P:(jc + 1) * P],
                            qT_sb[bp:bp + D, hp, i0 + c0:i0 + 512],
                            start=True,
                            stop=True,
                            skip_group_check=True,
                        )
                    ex = ph2.tile([P, 2, 512], f16, tag="ex", name="ex")
                    nc.scalar.activation(
                        ex[:, :, c0:512],
                        sps[:, :, c0:512],
                        ACTF.Exp,
                        scale=float(D) ** -0.5,
                    )
                    if diag:
                        nc.gpsimd.tensor_tensor(
                            ex[:, :, c0:c0 + P],
                            ex[:, :, c0:c0 + P],
                            tri_v[:, None, :].to_broadcast([P, 2, P]),
                            ALU.mult,
                        )
                    w = 512 - c0
                    clk["pe"] += w / 2.4 + 110
                    clk["act"] += 2 * w / 1.2 + 300
                    if diag:
                        clk["act"] += 300
                    return (jc, ex, c0)

                def emit_av_pair(pair):
                    for hi, av in ((0, av0), (1, av1)):
                        for jc, ex, c0 in pair:
                            nc.tensor.matmul(
                                av[:, c0:512],
                                v_sb[:, jc, hp, hi, :],
                                ex[:, hi, c0:512],
                                start=(jc == 0),
                                stop=(jc == njc - 1),
                                skip_group_check=True,
                            )
                            clk["pe"] += (512 - c0) / 2.4 + 60

                # chunk pairs: both scores, then (lagged by one pair) both AV
                # quads; fillers pad the PE to the ACT rate in between
                pend_pair = None
                npairs = njc // 2
                for p in range(npairs):
                    if ci >= 1 and p == 2 * ci:
                        drain_ph1(ci)
                    s0 = emit_s(2 * p)
                    s1 = emit_s(2 * p + 1)
                    if (ci, hp) != (0, 0) or p >= 1:
                        # steady drip: the phase is globally PE-bound, so one
                        # filler per pair spreads the backlog evenly; the
                        # rate check tops up if ACT would outrun the PE
                        fill(1, max_q=ci + 1)
                        fill_to_rate(max_q=ci + 1)
                    if pend_pair is not None:
                        emit_av_pair(pend_pair)
                    if pending is not None and p == 1:
                        # after the AV quad: the sel broadcast is a K=128
                        # matmul, adjacent K=128 work avoids two PE 64<->128
                        # stationary-mode switches
                        emit_norm(*pending)
                        pending = None
                        clk["pe"] += 230.0
                    pend_pair = (s0, s1)
                emit_av_pair(pend_pair)
                # drain av psum to SBUF (unnormalized, den rows embedded) so
                # the single-buffered av banks are free for the next block;
                # split across DVE and GpSimd so both finish in ~0.8us
                yU0 = pu.tile([P, 512], f16, tag="yU0", name="yU0")
                yU1 = pu.tile([P, 512], f16, tag="yU1", name="yU1")
                nc.vector.tensor_copy(yU0[:, :], av0[:, :])
                nc.scalar.copy(yU1[:, :], av1[:, :])
                nc.vector.tensor_copy(dsb[D:D + 1, :], yU0[D:D + 1, :])
                nc.vector.tensor_copy(dsb[0:1, :], yU1[0:1, :])
                pending = (hp, i0, yU0, yU1)
        # final block: normalize inline, then drain all remaining work
        hp, i0, yU0, yU1 = pending
        bps = ps_p.tile([P, 512], f32, tag="p", name="bps_f")
        nc.tensor.matmul(
            bps[:, :], sel_v, dsb[:],
            start=True, stop=True, skip_group_check=True,
        )
        rec = ph2.tile([P, 512], f32, tag="rec", bufs=2, name="rec_f")
        nc.vector.reciprocal_approx_fast(rec[:, :], bps[:, :])
        # per-128-slice so each projection tile unblocks as soon as its yT
        # columns are scaled, instead of serializing norm -> all projs
        for sl in range(4):
            s0, s1 = sl * P, (sl + 1) * P
            # norm multiplies on GpSimd: DVE and ACT are saturated by the
            # tail psum->sbuf copies, the Pool engine is free
            nc.gpsimd.tensor_tensor(
                yT_sb[0:D, hp, i0 + s0:i0 + s1], yU0[0:D, s0:s1],
                rec[0:D, s0:s1], ALU.mult,
            )
            nc.gpsimd.tensor_tensor(
                yT_sb[D:P, hp, i0 + s0:i0 + s1], yU1[D:P, s0:s1],
                rec[D:P, s0:s1], ALU.mult,
            )
            emit_proj_tj(12 + sl, tail=True)
        while proj_fill:
            proj_fill.pop(0)()
    nc.compile()
    return nc


def _get_nc():
    global _NC
    if _NC is None:
        _NC = _build_nc()
    return _NC


def _pack_inputs(x_b, W_qkv, b_qkv, W_proj, g):
    """Host-side packing for core (batch, head-group g): fp16, DMA-friendly."""
    f16 = np.float16
    s0 = HD * g
    xt = np.ascontiguousarray(x_b.T).astype(f16)          # [C, T]
    xqa = np.ascontiguousarray(
        xt.reshape(CC, P, 4, 512).transpose(1, 2, 0, 3)   # [p, quarter, o, t]
    )

    def wpack(col0):
        w = W_qkv[:, col0:col0 + HD].astype(f16)          # [C, HD]
        return np.ascontiguousarray(w.reshape(CC, P, 2, P).transpose(1, 2, 0, 3))

    wv_ = W_qkv[:, 2 * C + s0:2 * C + s0 + HD].astype(f16)
    wv_p = np.ascontiguousarray(wv_.reshape(CC, P, HD).transpose(1, 0, 2))
    wp_ = W_proj[s0:s0 + HD, :].astype(f16)               # [HD, C]
    wp_p = np.ascontiguousarray(wp_.reshape(2, P, C).transpose(1, 0, 2))

    cstm = np.zeros((P, CSTW), dtype=f16)
    cstm[:, OFF_TRI:OFF_TRI + P] = np.triu(np.ones((P, P), dtype=f16))
    cstm[D, OFF_SEL:OFF_SEL + D] = 1.0
    cstm[0, OFF_SEL + D:OFF_SEL + P] = 1.0
    cstm[:, OFF_BQ:OFF_BQ + 2] = b_qkv[s0:s0 + HD].reshape(2, P).T
    cstm[:, OFF_BK:OFF_BK + 2] = (
        b_qkv[C + s0:C + s0 + HD].reshape(2, P).T
    )
    cstm[:, OFF_BV:OFF_BV + HD] = b_qkv[2 * C + s0:2 * C + s0 + HD]

    return {
        "xq": xqa,
        "wq2": wpack(s0),
        "wk2": wpack(C + s0),
        "wv2": wv_p,
        "wp2": wp_p,
        "cst": np.ascontiguousarray(cstm),
    }


def kernel(x, W_qkv, b_qkv, W_proj, b_proj):
    global LAST_RESULTS
    from concourse import bass_utils

    x = np.asarray(x, dtype=np.float32)
    W_qkv = np.asarray(W_qkv, dtype=np.float32)
    b_qkv = np.asarray(b_qkv, dtype=np.float32)
    W_proj = np.asarray(W_proj, dtype=np.float32)
    b_proj = np.asarray(b_proj, dtype=np.float32)

    nc = _get_nc()
    in_maps = []
    for c in range(8):
        b, g = divmod(c, 4)
        in_maps.append(_pack_inputs(x[b], W_qkv, b_qkv, W_proj, g))

    res = bass_utils.run_bass_kernel_spmd(nc, in_maps, core_ids=list(range(8)))
    LAST_RESULTS = res
    ys = []
    for b in range(2):
        y = res.results[4 * b]["out"].astype(np.float64)
        for g in range(1, 4):
            y = y + res.results[4 * b + g]["out"]
        ys.append((y + b_proj).astype(np.float32))
    return np.stack(ys, axis=0)


# revision 25
# speedup vs baseline: 1.0100x; 1.0100x over previous
"""Causal self-attention (B=2, T=2048, C=1024, H=16) on 8 Trainium2 cores.

Sharding: data-parallel over batch (2) x tensor-parallel over heads (4 groups
of 4 heads). Core c handles batch b = c//4, head group g = c%4 (heads 4g..4g+3).
Each core computes its qkv column slice, full causal TxT attention for its 4
heads, and a partial row-parallel projection. Host sums the 4 partial proj
outputs per batch and adds b_proj.

v4 layout notes (on top of the v3 design):
- PSUM partitioned so the score double-buffer is never polluted:
  "s" scores [128,2,512] f32 x2 bufs (4 banks), av0/av1 accumulators x1 buf
  (2 banks), and a shared transient pool "p" [128,512] f32 x2 bufs (2 banks)
  that qkv/v/proj/bcast matmuls rotate through. In v3 proj tiles rotated
  through the score tag, which made every score matmul wait on the previous
  chunk's exp drain (measured 547ns/chunk of ACT idle).
- av bufs=1: at block end av0/av1 are drained to SBUF (yU tiles, f16,
  unnormalized + embedded den rows) split across DVE and GpSimd so the next
  block's first AV accumulate (start=True) finds the banks free.
- normalization (lagged one block): den rows -> dsb, sel-matrix broadcast
  matmul (into "p"), reciprocal_approx_fast, two [64,512] multiplies reading
  the SBUF yU tiles into yT.
- chunks processed in PAIRS: score(jc), score(jc+1) back-to-back, then both
  AV quads chained. Halves the PE 64<->128-row mode switches (~107ns each)
  and keeps ACT exps back-to-back.
- proj psum->sbuf copies ride GpSimd; DVE keeps bias adds, tri-mask, drain
  copies, reciprocal, norm multiplies.
- DMA: cc-granular first transfers so the first qkv chain starts ~4us in.
"""

import os
import sys

sys.path.insert(0, "/opt/trn_rl_repo")

import numpy as np

P = 128
T = 2048
C = 1024
D = 64
HPC = 4          # heads per core
HD = HPC * D     # 256 qkv columns per core
CC = C // P      # 8 contraction chunks
TC = T // P      # 16 t-chunks of 128
IC = T // 512    # 4 i-chunks of 512

# const blob column offsets
OFF_TRI = 0
OFF_SEL = 128
OFF_BQ = 256
OFF_BK = 258
OFF_BV = 260
CSTW = 516

_NC = None
LAST_RESULTS = None


def _build_nc():
    import concourse.mybir as mybir
    import concourse.tile as tile
    from concourse import bacc
    from contextlib import ExitStack

    dt = mybir.dt
    f32 = dt.float32
    f16 = dt.float16
    ALU = mybir.AluOpType
    ACTF = mybir.ActivationFunctionType

    nc = bacc.Bacc(
        "TRN2",
        target_bir_lowering=False,
        debug=False,
        enable_asserts=False,
        num_devices=8,
    )

    # host-packed layouts: contiguous per-partition lines per transfer
    xq = nc.dram_tensor("xq", [P, 4, CC, 512], f16, kind="ExternalInput").ap()
    wq2 = nc.dram_tensor("wq2", [P, 2, CC, P], f16, kind="ExternalInput").ap()
    wk2 = nc.dram_tensor("wk2", [P, 2, CC, P], f16, kind="ExternalInput").ap()
    wv2 = nc.dram_tensor("wv2", [P, CC, HD], f16, kind="ExternalInput").ap()
    wp2 = nc.dram_tensor("wp2", [P, 2, C], f16, kind="ExternalInput").ap()
    cst = nc.dram_tensor("cst", [P, CSTW], f16, kind="ExternalInput").ap()
    out = nc.dram_tensor("out", [T, C], f16, kind="ExternalOutput").ap()

    with tile.TileContext(nc) as tc, ExitStack() as ctx:
        persist = ctx.enter_context(tc.tile_pool(name="persist", bufs=1))
        qT_sb = persist.tile([P, 2, T], f16, name="qT")    # [d%128, dchunk, t]
        kT_sb = persist.tile([P, 2, T], f16, name="kT")
        v_sb = persist.tile([P, TC, 2, 2, P], f16, name="v")  # [t%128, tchunk, hpair, hi, 128-padded d]
        yT_sb = persist.tile([P, 2, T], f16, name="yT")
        wp_sb = persist.tile([P, 2, C], f16, name="wps")
        cst_sb = persist.tile([P, CSTW], f16, name="csts")
        dsb = persist.tile([P, 512], f16, name="dsb")
        xs_sb = persist.tile([P, 4, CC, 512], f16, name="xss")
        wq_sb = persist.tile([P, 2, CC, P], f16, name="wqs")
        wk_sb = persist.tile([P, 2, CC, P], f16, name="wks")
        wv_sb = persist.tile([P, CC, HD], f16, name="wvs")

        tri_v = cst_sb[:, OFF_TRI:OFF_TRI + P]
        sel_v = cst_sb[:, OFF_SEL:OFF_SEL + P]
        bq_v = cst_sb[:, OFF_BQ:OFF_BQ + 2]
        bk_v = cst_sb[:, OFF_BK:OFF_BK + 2]
        bv_v = cst_sb[:, OFF_BV:OFF_BV + HD].rearrange(
            "p (hp hi d) -> p hp hi d", hi=2, d=D
        )

        ph2 = ctx.enter_context(tc.tile_pool(name="ph2", bufs=7))
        ph3 = ctx.enter_context(tc.tile_pool(name="ph3", bufs=3))
        pu = ctx.enter_context(tc.tile_pool(name="pu", bufs=2))
        ps_s = ctx.enter_context(tc.tile_pool(name="ps_s", bufs=2, space="PSUM"))
        ps_av = ctx.enter_context(tc.tile_pool(name="ps_av", bufs=1, space="PSUM"))
        ps_p = ctx.enter_context(tc.tile_pool(name="ps_p", bufs=2, space="PSUM"))

        # one-time inits (no inputs needed)
        nc.vector.memset(dsb[:, :], 0.0)
        # constant regions of the padded v operand: zeros + the ones column
        # that makes each AV matmul emit the softmax denominator row
        nc.vector.memset(v_sb[:, :, :, 0, D + 1:P], 0.0)
        nc.vector.memset(v_sb[:, :, :, 1, 1:D], 0.0)
        nc.vector.memset(v_sb[:, :, :, 0, D:D + 1], 1.0)
        nc.vector.memset(v_sb[:, :, :, 1, 0:1], 1.0)

        # ---------------- DMA, first-needed-first ----------------
        # each dma_start costs ~0.6-0.8us of sequencer descriptor-generation
        # (DIRECT2D), so keep descriptors COARSE and spread across the three
        # trigger queues; one transfer fans out over all 16 DMA engines
        nc.sync.dma_start(xs_sb[:, 0, 0:1, :], xq[:, 0, 0:1, :])
        nc.scalar.dma_start(wq_sb[:, 0, 0:2, :], wq2[:, 0, 0:2, :])
        nc.gpsimd.dma_start(wk_sb[:, 0, :, :], wk2[:, 0, :, :])
        nc.sync.dma_start(xs_sb[:, 0, 1:4, :], xq[:, 0, 1:4, :])
        nc.scalar.dma_start(wq_sb[:, 0, 2:8, :], wq2[:, 0, 2:8, :])
        nc.sync.dma_start(xs_sb[:, 0, 4:8, :], xq[:, 0, 4:8, :])
        nc.scalar.dma_start(wq_sb[:, 1, :, :], wq2[:, 1, :, :])
        nc.gpsimd.dma_start(wv_sb[:], wv2)
        nc.sync.dma_start(cst_sb[:], cst)
        nc.gpsimd.dma_start(wk_sb[:, 1, :, :], wk2[:, 1, :, :])
        for tq in range(1, 4):
            nc.sync.dma_start(xs_sb[:, tq, 0:4, :], xq[:, tq, 0:4, :])
            nc.gpsimd.dma_start(xs_sb[:, tq, 4:8, :], xq[:, tq, 4:8, :])
        nc.gpsimd.dma_start(wp_sb[:], wp2)

        # ---------------- phase 1 units ----------------
        def emit_qk(W_s, bco, dest, co, tsl):
            ps = ps_p.tile([P, 512], f32, tag="p", name=f"qkp{co}_{tsl}")
            for cc in range(CC):
                nc.tensor.matmul(
                    ps[:, :],
                    W_s[:, co, cc, :],
                    xs_sb[:, tsl, cc, :],
                    start=(cc == 0),
                    stop=(cc == CC - 1),
                )
            nc.vector.tensor_tensor(
                dest[:, co, tsl * 512:(tsl + 1) * 512],
                ps[:, :],
                bco.to_broadcast([P, 512]),
                ALU.add,
            )

        def emit_v(tj):
            ps = ps_p.tile([P, 512], f32, tag="p", name=f"vp{tj}")
            for cc in range(CC):
                nc.tensor.matmul(
                    ps[:, 0:HD],
                    xs_sb[:, tj // 4, cc, (tj % 4) * P:(tj % 4 + 1) * P],
                    wv_sb[:, cc, :],
                    start=(cc == 0),
                    stop=(cc == CC - 1),
                )
            psv = ps[:, 0:HD].rearrange("p (hp hi d) -> p hp hi d", hi=2, d=D)
            nc.vector.tensor_tensor(
                v_sb[:, tj, :, 0, 0:D], psv[:, :, 0, :], bv_v[:, :, 0, :], ALU.add
            )
            nc.vector.tensor_tensor(
                v_sb[:, tj, :, 1, D:P], psv[:, :, 1, :], bv_v[:, :, 1, :], ALU.add
            )

        # prologue: only what block (0, hp=0) needs — q/k co0 + v 0-3; the
        # co1 chains ride the filler queue and drain during block (0,0)
        emit_qk(wq_sb, cst_sb[:, OFF_BQ:OFF_BQ + 1], qT_sb, 0, 0)
        emit_qk(wk_sb, cst_sb[:, OFF_BK:OFF_BK + 1], kT_sb, 0, 0)
        for tj in range(4):
            emit_v(tj)

        # quarters 1-3 (and quarter-0 co1) become attention-phase PE filler,
        # tagged with the t-quarter so blocks that need them can force-drain
        ph1_fill = []
        for W_s, boff, dest, kind in (
            (wq_sb, OFF_BQ, qT_sb, "q"),
            (wk_sb, OFF_BK, kT_sb, "k"),
        ):
            ph1_fill.append((0, kind, 1950.0, lambda
                W_s=W_s, boff=boff, dest=dest:
                emit_qk(W_s, cst_sb[:, boff + 1:boff + 2], dest, 1, 0)))
        for tsl in (1, 2, 3):
            for W_s, boff, dest, kind in (
                (wq_sb, OFF_BQ, qT_sb, "q"),
                (wk_sb, OFF_BK, kT_sb, "k"),
            ):
                for co in range(2):
                    ph1_fill.append((tsl, kind, 1950.0, lambda
                        W_s=W_s, boff=boff, dest=dest, co=co, tsl=tsl:
                        emit_qk(
                            W_s, cst_sb[:, boff + co:boff + co + 1],
                            dest, co, tsl,
                        )))
            for tj in range(4 * tsl, 4 * tsl + 4):
                ph1_fill.append((tsl, "v", 1300.0, lambda tj=tj: emit_v(tj)))
        proj_fill = []

        # ---------------- phase 2: attention + woven proj ----------------
        def emit_proj_tj(tj, tail=False):
            ot = ph3.tile([P, C], f16, tag="ot", name=f"ot{tj}")
            for co in range(2):
                # in the tail the score banks are free: a 4-slot psum
                # rotation keeps the proj matmuls ahead of the copies
                if tail and co == 0:
                    pps = ps_s.tile([P, 512], f32, tag="s", name=f"pp{tj}_{co}")
                else:
                    pps = ps_p.tile([P, 512], f32, tag="p", name=f"pp{tj}_{co}")
                for dc in range(2):
                    nc.tensor.matmul(
                        pps[:, :],
                        yT_sb[:, dc, tj * P:(tj + 1) * P],
                        wp_sb[:, dc, co * 512:(co + 1) * 512],
                        start=(dc == 0),
                        stop=(dc == 1),
                    )
                # in the tail ACT has no exps left: split the copies so the
                # last tiles drain twice as fast, and DMA each half out as
                # soon as its copy lands
                if tail and co == 1:
                    nc.scalar.copy(ot[:, co * 512:(co + 1) * 512], pps[:, :])
                else:
                    nc.vector.tensor_copy(
                        ot[:, co * 512:(co + 1) * 512], pps[:, :]
                    )
                if tail:
                    nc.sync.dma_start(
                        out[tj * P:(tj + 1) * P, co * 512:(co + 1) * 512],
                        ot[:, co * 512:(co + 1) * 512],
                    )
            if not tail:
                nc.sync.dma_start(out[tj * P:(tj + 1) * P, :], ot[:])

        # emitted-work accounting (ns) to pace fillers: the PE stream should
        # stay at least as long as the ACT (exp) stream it depends on
        clk = {"pe": 0.0, "act": 0.0}

        def fill(n, max_q=4):
            # max_q: don't pull qkv chains whose x-quarter DMA is still in
            # flight — a DMA-blocked filler matmul stalls the in-order PE
            # queue ahead of the next score matmuls
            for _ in range(n):
                if ph1_fill and ph1_fill[0][0] <= max_q:
                    q, kind, est, fn = ph1_fill.pop(0)
                    fn()
                    clk["pe"] += est
                elif proj_fill:
                    proj_fill.pop(0)()
                    clk["pe"] += 1250.0
                else:
                    return

        def fill_to_rate(max_q=4):
            while (ph1_fill or proj_fill) and clk["act"] > clk["pe"]:
                n0 = len(ph1_fill) + len(proj_fill)
                fill(1, max_q=max_q)
                if len(ph1_fill) + len(proj_fill) == n0:
                    return

        def drain_ph1(upto_quarter, kinds=("q", "k", "v")):
            i = 0
            while i < len(ph1_fill):
                q, kind, est, fn = ph1_fill[i]
                if q <= upto_quarter and kind in kinds:
                    ph1_fill.pop(i)
                    fn()
                    clk["pe"] += est
                else:
                    i += 1

        def emit_norm(hp, i0, yU0, yU1):
            # den rows were copied into dsb at the block's end; broadcast +
            # reciprocal + scale the SBUF yU tiles into yT
            bps = ps_p.tile([P, 512], f32, tag="p", name="bps")
            nc.tensor.matmul(
                bps[:, :], sel_v, dsb[:],
                start=True, stop=True, skip_group_check=True,
            )
            rec = ph2.tile([P, 512], f32, tag="rec", bufs=2, name="rec")
            nc.vector.reciprocal_approx_fast(rec[:, :], bps[:, :])
            nc.gpsimd.tensor_tensor(
                yT_sb[0:D, hp, i0:i0 + 512], yU0[0:D, :], rec[0:D, :], ALU.mult
            )
            nc.gpsimd.tensor_tensor(
                yT_sb[D:P, hp, i0:i0 + 512], yU1[D:P, :], rec[D:P, :], ALU.mult
            )
            if hp == 1 and i0 < 1536:
                ci = i0 // 512
                for tj in range(4 * ci, 4 * ci + 4):
                    proj_fill.append(lambda tj=tj: emit_proj_tj(tj))

        pending = None
        for ci in range(IC):
            i0 = ci * 512
            njc = 4 * (ci + 1)
            if ci >= 1:
                # only this i-chunk's qT is needed before the block starts;
                # its kT/v stragglers can drain any time before the diagonal
                drain_ph1(ci, kinds=("q",))
            for hp in range(2):
                if ci == 0 and hp == 1:
                    # quarter-0 co1 chains must be emitted before this
                    # block's scores read qT/kT co1
                    drain_ph1(0)
                av0 = ps_av.tile([P, 512], f32, tag="av0", name="av0")
                av1 = ps_av.tile([P, 512], f32, tag="av1", name="av1")

                def emit_s(jc):
                    diag = jc >= 4 * ci
                    o = (jc - 4 * ci) if diag else 0
                    c0 = o * P
                    sps = ps_s.tile([P, 2, 512], f32, tag="s", name="sps")
                    for hi in range(2):
                        bp = D * hi
                        nc.tensor.matmul(
                            sps[:, hi, c0:512],
                            kT_sb[bp:bp + D, hp, jc * P:(jc + 1) * P],
                            qT_sb[bp:bp + D, hp, i0 + c0:i0 + 512],
                            start=True,
                            stop=True,
                            skip_group_check=True,
                        )
                    ex = ph2.tile([P, 2, 512], f16, tag="ex", name="ex")
                    nc.scalar.activation(
                        ex[:, :, c0:512],
                        sps[:, :, c0:512],
                        ACTF.Exp,
                        scale=float(D) ** -0.5,
                    )
                    if diag:
                        # alternate engines so the two masks of a pair run
                        # in parallel and never serialize the exp->AV path
                        eng = nc.gpsimd if jc % 2 == 0 else nc.vector
                        eng.tensor_tensor(
                            ex[:, :, c0:c0 + P],
                            ex[:, :, c0:c0 + P],
                            tri_v[:, None, :].to_broadcast([P, 2, P]),
                            ALU.mult,
                        )
                    w = 512 - c0
                    clk["pe"] += w / 2.4 + 110
                    clk["act"] += 2 * w / 1.2 + 300
                    if diag:
                        clk["act"] += 300
                    return (jc, ex, c0)

                def emit_av_pair(pair):
                    for hi, av in ((0, av0), (1, av1)):
                        for jc, ex, c0 in pair:
                            nc.tensor.matmul(
                                av[:, c0:512],
                                v_sb[:, jc, hp, hi, :],
                                ex[:, hi, c0:512],
                                start=(jc == 0),
                                stop=(jc == njc - 1),
                                skip_group_check=True,
                            )
                            clk["pe"] += (512 - c0) / 2.4 + 60

                # chunk pairs: both scores, then (lagged by up to two pairs)
                # the AV matmuls in 8-long K=128 chains; fillers pad the PE
                # to the ACT rate in between
                pend_pairs = []
                npairs = njc // 2
                for p in range(npairs):
                    if ci >= 1 and p == 2 * ci:
                        drain_ph1(ci)
                    s0 = emit_s(2 * p)
                    s1 = emit_s(2 * p + 1)
                    if (ci, hp) != (0, 0) or p >= 1:
                        # steady drip: the phase is globally PE-bound, so one
                        # filler per pair spreads the backlog evenly; the
                        # rate check tops up if ACT would outrun the PE
                        fill(1, max_q=ci + 1)
                        fill_to_rate(max_q=ci + 1)
                    if len(pend_pairs) == 2:
                        for pp in pend_pairs:
                            emit_av_pair(pp)
                        pend_pairs = []
                    if pending is not None and p == 1:
                        # after the AV chains: the sel broadcast is a K=128
                        # matmul, adjacent K=128 work avoids two PE 64<->128
                        # stationary-mode switches
                        emit_norm(*pending)
                        pending = None
                        clk["pe"] += 230.0
                    pend_pairs.append((s0, s1))
                for pp in pend_pairs:
                    emit_av_pair(pp)
                # drain av psum to SBUF (unnormalized, den rows embedded) so
                # the single-buffered av banks are free for the next block;
                # split across DVE and GpSimd so both finish in ~0.8us
                yU0 = pu.tile([P, 512], f16, tag="yU0", name="yU0")
                yU1 = pu.tile([P, 512], f16, tag="yU1", name="yU1")
                nc.vector.tensor_copy(yU0[:, :], av0[:, :])
                nc.scalar.copy(yU1[:, :], av1[:, :])
                nc.vector.tensor_copy(dsb[D:D + 1, :], yU0[D:D + 1, :])
                nc.vector.tensor_copy(dsb[0:1, :], yU1[0:1, :])
                pending = (hp, i0, yU0, yU1)
        # final block: normalize inline, then drain all remaining work
        hp, i0, yU0, yU1 = pending
        bps = ps_p.tile([P, 512], f32, tag="p", name="bps_f")
        nc.tensor.matmul(
            bps[:, :], sel_v, dsb[:],
            start=True, stop=True, skip_group_check=True,
        )
        rec = ph2.tile([P, 512], f32, tag="rec", bufs=2, name="rec_f")
        # per-128-slice (including the reciprocal) so each projection tile
        # unblocks as soon as its yT columns are scaled, instead of
        # serializing norm -> all projs
        for sl in range(4):
            s0, s1 = sl * P, (sl + 1) * P
            nc.vector.reciprocal_approx_fast(rec[:, s0:s1], bps[:, s0:s1])
            # norm multiplies on GpSimd: DVE and ACT are saturated by the
            # tail psum->sbuf copies, the Pool engine is free
            nc.gpsimd.tensor_tensor(
                yT_sb[0:D, hp, i0 + s0:i0 + s1], yU0[0:D, s0:s1],
                rec[0:D, s0:s1], ALU.mult,
            )
            nc.gpsimd.tensor_tensor(
                yT_sb[D:P, hp, i0 + s0:i0 + s1], yU1[D:P, s0:s1],
                rec[D:P, s0:s1], ALU.mult,
            )
            emit_proj_tj(12 + sl, tail=True)
        while proj_fill:
            proj_fill.pop(0)()
    nc.compile()
    return nc


def _get_nc():
    global _NC
    if _NC is None:
        _NC = _build_nc()
    return _NC


def _pack_inputs(x_b, W_qkv, b_qkv, W_proj, g):
    """Host-side packing for core (batch, head-group g): fp16, DMA-friendly."""
    f16 = np.float16
    s0 = HD * g
    xt = np.ascontiguousarray(x_b.T).astype(f16)          # [C, T]
    xqa = np.ascontiguousarray(
        xt.reshape(CC, P, 4, 512).transpose(1, 2, 0, 3)   # [p, quarter, o, t]
    )

    def wpack(col0):
        w = W_qkv[:, col0:col0 + HD].astype(f16)          # [C, HD]
        return np.ascontiguousarray(w.reshape(CC, P, 2, P).transpose(1, 2, 0, 3))

    wv_ = W_qkv[:, 2 * C + s0:2 * C + s0 + HD].astype(f16)
    wv_p = np.ascontiguousarray(wv_.reshape(CC, P, HD).transpose(1, 0, 2))
    wp_ = W_proj[s0:s0 + HD, :].astype(f16)               # [HD, C]
    wp_p = np.ascontiguousarray(wp_.reshape(2, P, C).transpose(1, 0, 2))

    cstm = np.zeros((P, CSTW), dtype=f16)
    cstm[:, OFF_TRI:OFF_TRI + P] = np.triu(np.ones((P, P), dtype=f16))
    cstm[D, OFF_SEL:OFF_SEL + D] = 1.0
    cstm[0, OFF_SEL + D:OFF_SEL + P] = 1.0
    cstm[:, OFF_BQ:OFF_BQ + 2] = b_qkv[s0:s0 + HD].reshape(2, P).T
    cstm[:, OFF_BK:OFF_BK + 2] = (
        b_qkv[C + s0:C + s0 + HD].reshape(2, P).T
    )
    cstm[:, OFF_BV:OFF_BV + HD] = b_qkv[2 * C + s0:2 * C + s0 + HD]

    return {
        "xq": xqa,
        "wq2": wpack(s0),
        "wk2": wpack(C + s0),
        "wv2": wv_p,
        "wp2": wp_p,
        "cst": np.ascontiguousarray(cstm),
    }


def kernel(x, W_qkv, b_qkv, W_proj, b_proj):
    global LAST_RESULTS
    from concourse import bass_utils

    x = np.asarray(x, dtype=np.float32)
    W_qkv = np.asarray(W_qkv, dtype=np.float32)
    b_qkv = np.asarray(b_qkv, dtype=np.float32)
    W_proj = np.asarray(W_proj, dtype=np.float32)
    b_proj = np.asarray(b_proj, dtype=np.float32)

    nc = _get_nc()
    in_maps = []
    for c in range(8):
        b, g = divmod(c, 4)
        in_maps.append(_pack_inputs(x[b], W_qkv, b_qkv, W_proj, g))

    res = bass_utils.run_bass_kernel_spmd(nc, in_maps, core_ids=list(range(8)))
    LAST_RESULTS = res
    ys = []
    for b in range(2):
        y = res.results[4 * b]["out"].astype(np.float64)
        for g in range(1, 4):
            y = y + res.results[4 * b + g]["out"]
        ys.append((y + b_proj).astype(np.float32))
    return np.stack(ys, axis=0)
